# revision 37
# baseline (speedup 1.0000x reference)
"""Trainium2 Bass kernel for nn_EmberBlock (dense transformer block with LIF-gated
attention). 8-core SPMD: head-parallel attention (4 heads/core, one batch per
4-core group) + sequence-parallel MLP after chunked bf16 ReduceScatters.

Attention runs in S^T (key-major) layout: scores are computed transposed so the
gated probabilities feed P·V directly as the moving operand — no per-tile PE
transposes of the attention matrix and no PSUM->SBUF copies for it. Softmax
column sums come from ones-vector matmuls; per-query normalizers are broadcast
back across partitions with rank-1 matmuls; the post-gate renormalizer rides a
ones-column appended to V and is applied while evacuating the PV PSUM.

kernel(**inputs) takes FULL unsharded inputs (as in reference.setup_inputs())
and returns the FULL [B, T, C] output.
"""
import numpy as np
import ml_dtypes

import concourse.bass as bass
import concourse.mybir as mybir
import concourse.tile as tile
from concourse import bacc
from concourse.bass_utils import run_bass_kernel_spmd

F32 = mybir.dt.float32
BF16 = mybir.dt.bfloat16
AF = mybir.ActivationFunctionType
ALU = mybir.AluOpType

# model dims (hardcoded per spec)
B, T, C = 2, 2048, 1024
H, D = 16, 64
FF = 4 * C                    # 4096
N_CORES = 8
GROUP = 4                     # cores per batch
HL = H // GROUP               # 4 local heads
LC = HL * D                   # 256 local head feature cols
EPS_LN = 1e-5
P = 128                       # partitions
NT = T // P                   # 16 token tiles per batch
NQS = 4                       # q-slabs of 512 tokens
MLP_TOK = 512                 # tokens per core in MLP phase (4 strips of 128)

_CACHED_NC = None


def _build():
    nc = bacc.Bacc(None, target_bir_lowering=False, debug=False, num_devices=N_CORES)

    # ---------------- I/O ----------------
    x_b = nc.dram_tensor("x_b", [T, C], BF16, kind="ExternalInput")
    x_res = nc.dram_tensor("x_res", [MLP_TOK, C], F32, kind="ExternalInput")
    wqkv = nc.dram_tensor("wqkv", [C, 3 * LC], BF16, kind="ExternalInput")
    bqkv = nc.dram_tensor("bqkv", [3 * LC], F32, kind="ExternalInput")
    wproj = nc.dram_tensor("wproj", [LC, C], BF16, kind="ExternalInput")
    bproj = nc.dram_tensor("bproj", [C], F32, kind="ExternalInput")
    wfc = nc.dram_tensor("wfc", [C, FF], BF16, kind="ExternalInput")
    bfc = nc.dram_tensor("bfc", [FF], F32, kind="ExternalInput")
    wmlp = nc.dram_tensor("wmlp", [FF, C], BF16, kind="ExternalInput")
    bmlp = nc.dram_tensor("bmlp", [C], F32, kind="ExternalInput")
    lif = nc.dram_tensor("lif", [4, HL], F32, kind="ExternalInput")
    out = nc.dram_tensor("out", [MLP_TOK, C], F32, kind="ExternalOutput")

    # RS bounce buffers (internal DRAM), bf16 to halve collective traffic
    rs_in = nc.dram_tensor("rs_in", [T, C], BF16)
    rs_out = nc.dram_tensor("rs_out", [MLP_TOK, C], BF16)

    # constants embedded in the NEFF
    id_bf = nc.inline_tensor(np.eye(P, dtype=ml_dtypes.bfloat16), name="id_bf")
    # causal mask for a diagonal 128x128 block: mask[r, c] = 1 iff c >= r
    mk = np.triu(np.ones((P, P), np.float32))
    mask_c = nc.inline_tensor(mk.astype(ml_dtypes.bfloat16), name="mask_c")

    replica_groups = [[0, 1, 2, 3], [4, 5, 6, 7]]

    from contextlib import ExitStack
    with tile.TileContext(nc) as tc, ExitStack() as root_ctx:
        attn_ctx = ExitStack()
        consts = root_ctx.enter_context(tc.tile_pool(name="consts", bufs=1))
        zero_c = consts.tile([P, 1], F32)
        nc.vector.memset(zero_c[:], 0.0)
        nc.const_aps.aps[(F32, 0.0)] = zero_c[:]
        eps_c = consts.tile([P, 1], F32)
        nc.vector.memset(eps_c[:], EPS_LN)
        nc.const_aps.aps[(F32, EPS_LN)] = eps_c[:]
        ident = consts.tile([P, P], BF16)
        nc.sync.dma_start(out=ident[:], in_=id_bf[:, :])
        masks = consts.tile([P, P], BF16)
        nc.sync.dma_start(out=masks[:], in_=mask_c[:, :])
        ones_col = consts.tile([P, 1], BF16)
        nc.vector.memset(ones_col[:], 1.0)
        ones_full = consts.tile([P, P], BF16)
        nc.vector.memset(ones_full[:], 1.0)
        ones_row = consts.tile([1, P], BF16)
        nc.vector.memset(ones_row[:], 1.0)
        # per-head LIF constants broadcast to all partitions: [128, 4, HL]
        lif_sb = consts.tile([P, 4, HL], F32)
        nc.sync.dma_start(out=lif_sb[:], in_=lif[None, :, :].to_broadcast((P, 4, HL)))
        # biases in per-partition layout
        bqkv_sb = consts.tile([P, 6], F32)
        nc.sync.dma_start(out=bqkv_sb[:], in_=bqkv.rearrange("(t p) -> p t", p=P))
        bfc_sb = consts.tile([P, FF // P], F32)
        nc.sync.dma_start(out=bfc_sb[:], in_=bfc.rearrange("(t p) -> p t", p=P))
        # attention-projection weights (2 k-tiles), small -> consts
        wproj_sb = consts.tile([P, 2, C], BF16)
        for kt in range(2):
            nc.sync.dma_start(out=wproj_sb[:, kt, :], in_=wproj[kt * P:(kt + 1) * P, :])

        # =========== Phase 1: LN1 over all T tokens + h^T ===========
        stat_pool = root_ctx.enter_context(tc.tile_pool(name="stats", bufs=4))
        mm_psum = root_ctx.enter_context(tc.tile_pool(name="mm_psum", bufs=2, space="PSUM"))
        wfcp1 = root_ctx.enter_context(tc.tile_pool(name="wfcp1", bufs=1))
        wfc_sb1 = wfcp1.tile([P, 8, FF // 2], BF16)
        attn = attn_ctx.enter_context(tc.tile_pool(name="attn", bufs=1))
        p12_ctx = ExitStack()
        p12 = p12_ctx.enter_context(tc.tile_pool(name="p12", bufs=1))
        xio = p12_ctx.enter_context(tc.tile_pool(name="xio", bufs=3))
        tp_ctx = ExitStack()
        tp_psum = tp_ctx.enter_context(tc.tile_pool(name="tp_psum", bufs=4, space="PSUM"))

        hT = p12.tile([P, C // P, T], BF16)       # [128, 8, 2048] feature-major h
        wqkv_sb = p12.tile([P, 8, 3 * LC], BF16)  # 8 k-tiles of wqkv
        for kt in range(8):
            nc.sync.dma_start(out=wqkv_sb[:, kt, :], in_=wqkv[kt * P:(kt + 1) * P, :])

        def layernorm_tile(x_tile, h_out, tag):
            """x_tile [128, C] -> h_out [128, C] bf16 (normalized, no affine)."""
            stats = stat_pool.tile([P, 2, 6], F32, name=f"st_{tag}")
            nc.vector.bn_stats(out=stats[:, 0, :], in_=x_tile[:, 0:512])
            nc.vector.bn_stats(out=stats[:, 1, :], in_=x_tile[:, 512:1024])
            mv = stat_pool.tile([P, 2], F32, name=f"mv_{tag}")
            nc.vector.bn_aggr(out=mv[:], in_=stats[:])
            std = stat_pool.tile([P, 1], F32, name=f"sd_{tag}")
            nc.scalar.activation(std[:], mv[:, 1:2], AF.Sqrt, bias=EPS_LN)
            rstd = stat_pool.tile([P, 1], F32, name=f"rs_{tag}")
            nc.vector.reciprocal(rstd[:], std[:])
            nmr = stat_pool.tile([P, 1], F32, name=f"nm_{tag}")
            nc.vector.tensor_scalar(out=nmr[:], in0=mv[:, 0:1], scalar1=rstd[:],
                                    scalar2=-1.0, op0=ALU.mult, op1=ALU.mult)
            nc.scalar.activation(h_out, x_tile, AF.Identity, bias=nmr[:], scale=rstd[:])

        qkvT = attn.tile([P, 6, T], BF16)   # rows: q(2 tiles) k(2) v(2)

        # Phases 1+2 interleaved per 512-token chunk: LN+transpose of 4 tiles,
        # then that chunk's QKV matmuls (keeps PE busy during DMA/LN latency)
        for ns in range(4):
            for tt in range(4 * ns, 4 * ns + 4):
                x_tile = xio.tile([P, C], BF16, name="x_t", tag="x_t")
                nc.sync.dma_start(out=x_tile[:], in_=x_b[tt * P:(tt + 1) * P, :])
                h_tile = xio.tile([P, C], BF16, name="h_t", tag="h_t")
                layernorm_tile(x_tile[:], h_tile[:], f"ln1_{tt}")
                # transpose h [128, C] -> hT[:, ft, tt*128:...]
                for fg in range(2):  # groups of 4 feature tiles -> one psum bank
                    tp = tp_psum.tile([P, 4, P], BF16, name="htp", tag="htp")
                    for j in range(4):
                        ft = fg * 4 + j
                        nc.tensor.transpose(tp[:, j, :], h_tile[:, ft * P:(ft + 1) * P],
                                            ident[:])
                    if fg == 0:
                        nc.scalar.activation(
                            hT[:, 0:4, tt * P:(tt + 1) * P], tp[:], AF.Identity)
                    else:
                        nc.vector.tensor_copy(
                            hT[:, 4:8, tt * P:(tt + 1) * P], tp[:])
            for mt in range(6):
                ps = mm_psum.tile([P, 512], F32, name="qkv_ps", tag="qkv_ps")
                for kt in range(8):
                    nc.tensor.matmul(ps[:],
                                     wqkv_sb[:, kt, mt * P:(mt + 1) * P],
                                     hT[:, kt, ns * 512:(ns + 1) * 512],
                                     start=(kt == 0), stop=(kt == 7))
                nc.scalar.activation(qkvT[:, mt, ns * 512:(ns + 1) * 512], ps[:],
                                     AF.Identity, bias=bqkv_sb[:, mt:mt + 1])

        # first wfc half preload (x loads are queued; overlaps attention)
        for kt in range(8):
            nc.sync.dma_start(out=wfc_sb1[:, kt, :],
                              in_=wfc[kt * P:(kt + 1) * P, 0:FF // 2])

        # =========== Phase 3: V^T -> V (token-major, +ones col for renorm) ===
        v1 = attn.tile([P, NT, HL, D + 1], BF16)   # [128, 16, 4, 65]
        nc.vector.memset(v1[:, :, :, D:D + 1], 1.0)
        for vt in range(2):
            for tg in range(4):  # 4 token tiles per psum bank
                tp = tp_psum.tile([P, 4, P], BF16, name="vtp", tag="htp")
                for j in range(4):
                    tt = tg * 4 + j
                    nc.tensor.transpose(tp[:, j, :],
                                        qkvT[:, 4 + vt, tt * P:(tt + 1) * P],
                                        ident[:])
                nc.vector.tensor_copy(
                    v1[:, tg * 4:(tg + 1) * 4, 2 * vt:2 * vt + 2, 0:D],
                    tp[:])
        tp_ctx.close()   # release transpose psum banks for attention
        p12_ctx.close()  # hT / wqkv dead after QKV

        # =========== Phase 4: attention, S^T (key-major) layout ===========
        epool = attn_ctx.enter_context(tc.tile_pool(name="epool", bufs=2))
        zpool = attn_ctx.enter_context(tc.tile_pool(name="zpool", bufs=1))
        rowp = attn_ctx.enter_context(tc.tile_pool(name="rowp", bufs=1))
        bcsb = attn_ctx.enter_context(tc.tile_pool(name="bcsb", bufs=2))
        psA = attn_ctx.enter_context(tc.tile_pool(name="psA", bufs=3, space="PSUM"))
        psSE = attn_ctx.enter_context(tc.tile_pool(name="psSE", bufs=2, space="PSUM"))
        psY = attn_ctx.enter_context(tc.tile_pool(name="psY", bufs=1, space="PSUM"))
        ypool = attn_ctx.enter_context(tc.tile_pool(name="ypool", bufs=2))
        o2pool = attn_ctx.enter_context(tc.tile_pool(name="o2pool", bufs=2))

        yTs = {}

        def stageA(qs, h):
            """S^T tiles + exp + column-sum accumulation for head h."""
            nkb, q0 = qs * 4 + 4, qs * 512
            qrow = (h % 2) * D
            qtile = h // 2
            ktile = 2 + h // 2
            eT = epool.tile([P, NT, 512], BF16, name=f"eT{h % 2}",
                            tag=f"eT{h % 2}")
            se = psSE.tile([P, 512], F32, name="se", tag="se")
            q_ap = qkvT[qrow:qrow + D, qtile, q0:q0 + 512]
            pend = []  # (kb, c0, psum) awaiting exp+sum

            def drain():
                kb, c0, ps = pend.pop(0)
                nc.scalar.activation(eT[:, kb, c0:], ps[:, c0:], AF.Exp)
                if kb >= qs * 4:  # diagonal tile: mask the 128-wide block
                    nc.vector.tensor_tensor(out=eT[:, kb, c0:c0 + P],
                                            in0=eT[:, kb, c0:c0 + P],
                                            in1=masks[:], op=ALU.mult)
                nc.tensor.matmul(se[:, c0:], ones_full[:], eT[:, kb, c0:],
                                 start=(kb == 0), stop=(kb == nkb - 1))

            for kb in range(nkb):
                c0 = max(0, kb * P - q0)
                ps = psA.tile([P, 512], F32, name="sps", tag="sps")
                nc.tensor.matmul(
                    ps[:, c0:],
                    qkvT[qrow:qrow + D, ktile, kb * P:(kb + 1) * P],
                    q_ap[:, c0:] if c0 else q_ap,
                    start=True, stop=True)
                pend.append((kb, c0, ps))
                if len(pend) > 2:
                    drain()
            while pend:
                drain()
            state[(qs, h)] = (eT, se)

        def stageB(qs, h):
            """1/se broadcast to all partitions (bf16)."""
            eT, se = state[(qs, h)]
            rse_f = bcsb.tile([P, 512], F32, name=f"rsef{h % 2}",
                              tag=f"rsef{h % 2}")
            nc.vector.reciprocal_approx_fast(rse_f[:], se[:])
            bc = bcsb.tile([P, 512], BF16, name=f"bc{h % 2}", tag=f"bc{h % 2}")
            nc.vector.tensor_copy(bc[:], rse_f[:])
            state[(qs, h)] = (eT, bc)

        def stageCDE(qs, h):
            """Gate, PV, renormalize into yT for head h."""
            nkb, q0 = qs * 4 + 4, qs * 512
            qrow = (h % 2) * D
            eT, bc = state.pop((qs, h))
            ensure_yT(qs)
            yT = yTs[qs]
            zf = zpool.tile([P, NT, 512], BF16, name=f"zf{h % 2}",
                            tag=f"zf{h % 2}")
            for kb in range(nkb):
                c0 = max(0, kb * P - q0)
                # p = e / se  (true softmax prob)
                nc.vector.tensor_tensor(out=zf[:, kb, c0:], in0=eT[:, kb, c0:],
                                        in1=bc[:, c0:], op=ALU.mult)
            # single big instructions over the whole slab (dead regions of
            # diagonal tiles hold garbage; PV skips them via c0 slices)
            # fire-gate: tanh((st/2) p - st th / 2)
            nc.scalar.activation(zf[:, 0:nkb, :], zf[:, 0:nkb, :], AF.Tanh,
                                 bias=lif_sb[:, 1, h:h + 1],
                                 scale=lif_sb[:, 0, h:h + 1])
            # w = c1 tanh + c0 ; m = w * e (renormalized later)
            nc.vector.tensor_scalar(out=zf[:, 0:nkb, :], in0=zf[:, 0:nkb, :],
                                    scalar1=lif_sb[:, 2, h:h + 1],
                                    scalar2=lif_sb[:, 3, h:h + 1],
                                    op0=ALU.mult, op1=ALU.add)
            nc.vector.tensor_tensor(out=zf[:, 0:nkb, :], in0=zf[:, 0:nkb, :],
                                    in1=eT[:, 0:nkb, :], op=ALU.mult)
            # PV with appended ones row: yps[0:64] = y^T, yps[64] = sum m
            yps = psY.tile([D + 1, 512], F32, name="yps", tag="yps")
            for kb in range(nkb):
                c0 = max(0, kb * P - q0)
                nc.tensor.matmul(yps[:, c0:], v1[:, kb, h, :],
                                 zf[:, kb, c0:],
                                 start=(kb == 0), stop=(kb == nkb - 1))
            # renormalize: yT = yps[0:64] / sum
            sm_s = rowp.tile([1, 512], F32, name="sms", tag="sms")
            nc.scalar.activation(sm_s[:], yps[D:D + 1, :], AF.Identity)
            rsm = rowp.tile([1, 512], F32, name="rsm", tag="rsm")
            nc.vector.reciprocal_approx_fast(rsm[:], sm_s[:])
            rsm_b = rowp.tile([1, 512], BF16, name="rsmb", tag="rsmb")
            nc.vector.tensor_copy(rsm_b[:], rsm[:])
            bc2_ps = mm_psum.tile([P, 512], F32, name="bc2", tag="qkv_ps")
            nc.tensor.matmul(bc2_ps[0:D, :], ones_row[:, 0:D], rsm_b[:],
                             start=True, stop=True)
            bc2 = bcsb.tile([P, 512], BF16, name=f"bc2{h % 2}",
                            tag=f"bc{h % 2}")
            nc.scalar.activation(bc2[0:D, :], bc2_ps[0:D, :], AF.Identity)
            nc.vector.tensor_tensor(out=yT[h // 2][qrow:qrow + D, :],
                                    in0=yps[0:D, :], in1=bc2[0:D, :],
                                    op=ALU.mult)

        def proj_rs(qs):
            """Attention projection + ReduceScatter chunk for slab qs."""
            q0 = qs * 512
            yT = yTs.pop(qs)
            for mt in range(4):
                o2 = o2pool.tile([P, C], BF16, name="o2", tag="o2")
                for ns in range(2):
                    ps = mm_psum.tile([P, 512], F32, name="o2_ps", tag="qkv_ps")
                    for kt in range(2):
                        nc.tensor.matmul(ps[:],
                                         yT[kt][:, mt * P:(mt + 1) * P],
                                         wproj_sb[:, kt, ns * 512:(ns + 1) * 512],
                                         start=(kt == 0), stop=(kt == 1))
                    nc.vector.tensor_copy(o2[:, ns * 512:(ns + 1) * 512], ps[:])
                nc.sync.dma_start(
                    out=rs_in[q0 + mt * P: q0 + (mt + 1) * P, :],
                    in_=o2[:])
            # chunked ReduceScatter: this slab's 512 rows -> 128 local rows
            nc.gpsimd.collective_compute(
                "ReduceScatter", ALU.add, replica_groups=replica_groups,
                ins=[rs_in[q0:q0 + 512, :]],
                outs=[rs_out[qs * P:(qs + 1) * P, :]])

        # continuous 16-step (slab, head) pipeline — no drain at slab bounds
        state = {}
        items = [(qs, h) for qs in reversed(range(NQS)) for h in range(HL)]
        def ensure_yT(qs):
            if qs not in yTs:
                yTs[qs] = [ypool.tile([P, 512], BF16, name=f"yT{i}_{qs}",
                                      tag=f"yT{i}") for i in range(2)]
        stageA(*items[0])
        for i in range(1, len(items)):
            stageB(*items[i - 1])
            stageA(*items[i])
            if i >= 3:
                stageCDE(*items[i - 3])
                if items[i - 3][1] == HL - 1:
                    proj_rs(items[i - 3][0])
        stageB(*items[-1])
        for i in (-3, -2, -1):
            stageCDE(*items[i])
            if items[i][1] == HL - 1:
                proj_rs(items[i][0])

        # release attention pools
        attn_ctx.close()

        # =========== Phase 5: MLP on 512 local tokens ===========
        wfcp2 = root_ctx.enter_context(tc.tile_pool(name="wfcp2", bufs=1))
        wfc_sb2 = wfcp2.tile([P, 8, FF // 2], BF16)
        for kt in range(8):
            nc.sync.dma_start(out=wfc_sb2[:, kt, :],
                              in_=wfc[kt * P:(kt + 1) * P, FF // 2:])
        mlp = root_ctx.enter_context(tc.tile_pool(name="mlp", bufs=1))
        wstream = root_ctx.enter_context(tc.tile_pool(name="wstream", bufs=2))
        tp2_psum = root_ctx.enter_context(tc.tile_pool(name="tp2", bufs=2,
                                                       space="PSUM"))
        o3_psum = root_ctx.enter_context(tc.tile_pool(name="o3p", bufs=2,
                                                      space="PSUM"))

        # free-dim biases broadcast across partitions
        bproj_sb = mlp.tile([P, C], BF16)
        nc.gpsimd.dma_start(out=bproj_sb[:], in_=bproj[None, :].to_broadcast((P, C)))
        bmlp_sb = mlp.tile([P, C], BF16)
        nc.gpsimd.dma_start(out=bmlp_sb[:], in_=bmlp[None, :].to_broadcast((P, C)))
        h2T = mlp.tile([P, 8, MLP_TOK], BF16)
        aT = mlp.tile([P, FF // P, MLP_TOK], BF16)
        x1_t = {}
        for half in (1, 0):
            for j in (2 * half, 2 * half + 1):
                rs_sb = mlp.tile([P, C], BF16, name=f"rs_sb{j}", tag="rs_sb")
                nc.sync.dma_start(out=rs_sb[:], in_=rs_out[j * P:(j + 1) * P, :])
                xr = mlp.tile([P, C], F32, name=f"xr{j}", tag="xr")
                nc.sync.dma_start(out=xr[:], in_=x_res[j * P:(j + 1) * P, :])
                x1 = mlp.tile([P, C], F32, name=f"x1_{j}")
                nc.vector.tensor_tensor(out=x1[:], in0=xr[:], in1=rs_sb[:],
                                        op=ALU.add)
                nc.vector.tensor_tensor(out=x1[:], in0=x1[:], in1=bproj_sb[:],
                                        op=ALU.add)
                x1_t[j] = x1
                h2 = mlp.tile([P, C], BF16, name=f"h2_{j}", tag="h2")
                layernorm_tile(x1[:], h2[:], f"ln2_{j}")
                for fg in range(2):
                    tp = tp2_psum.tile([P, 4, P], BF16, name="h2tp", tag="h2tp")
                    for k in range(4):
                        ft = fg * 4 + k
                        nc.tensor.transpose(tp[:, k, :], h2[:, ft * P:(ft + 1) * P],
                                            ident[:])
                    nc.vector.tensor_copy(
                        h2T[:, fg * 4:(fg + 1) * 4, j * P:(j + 1) * P], tp[:])
            # FC + gelu on this 256-token half
            for mt in range(FF // P):
                wsb = wfc_sb1 if mt < 16 else wfc_sb2
                mt0 = mt if mt < 16 else mt - 16
                ps = o3_psum.tile([P, 256], F32, name="fc_ps", tag="o3_ps")
                for kt in range(8):
                    nc.tensor.matmul(ps[:], wsb[:, kt, mt0 * P:(mt0 + 1) * P],
                                     h2T[:, kt, half * 256:(half + 1) * 256],
                                     start=(kt == 0), stop=(kt == 7))
                nc.scalar.activation(aT[:, mt, half * 256:(half + 1) * 256], ps[:],
                                     AF.Gelu, bias=bfc_sb[:, mt:mt + 1])

        # MLP proj + residual accumulated in place into x1
        for ns in range(4):
            wm_q = wstream.tile([P, FF // P, 256], BF16, name="wm_q", tag="wm_q")
            for kt in range(FF // P):
                nc.sync.dma_start(out=wm_q[:, kt, :],
                                  in_=wmlp[kt * P:(kt + 1) * P,
                                           ns * 256:(ns + 1) * 256])
            for j in range(4):
                ps = o3_psum.tile([P, 256], F32, name="o3_ps", tag="o3_ps")
                for kt in range(FF // P):
                    nc.tensor.matmul(ps[:], aT[:, kt, j * P:(j + 1) * P],
                                     wm_q[:, kt, :],
                                     start=(kt == 0), stop=(kt == FF // P - 1))
                nc.vector.tensor_tensor(out=x1_t[j][:, ns * 256:(ns + 1) * 256],
                                        in0=x1_t[j][:, ns * 256:(ns + 1) * 256],
                                        in1=ps[:], op=ALU.add)
        for j in range(4):
            nc.vector.tensor_tensor(out=x1_t[j][:], in0=x1_t[j][:],
                                    in1=bmlp_sb[:], op=ALU.add)
            nc.sync.dma_start(out=out[j * P:(j + 1) * P, :], in_=x1_t[j][:])

    nc.compile()
    return nc


def _get_nc():
    global _CACHED_NC
    if _CACHED_NC is None:
        _CACHED_NC = _build()
    return _CACHED_NC


def _softplus(x):
    return np.log1p(np.exp(-np.abs(x))) + np.maximum(x, 0.0)


def _bf16(x):
    return np.ascontiguousarray(x.astype(ml_dtypes.bfloat16))


def kernel(x, ln1_w, ln1_b, w_attn, b_attn, w_attn_proj, b_attn_proj,
           threshold, leak, steepness, ln2_w, ln2_b,
           w_fc, b_fc, w_mlp_proj, b_mlp_proj):
    x = np.asarray(x, np.float32)
    f32 = lambda a: np.asarray(a, np.float32)
    ln1_w, ln1_b, w_attn, b_attn = map(f32, (ln1_w, ln1_b, w_attn, b_attn))
    w_attn_proj, b_attn_proj = f32(w_attn_proj), f32(b_attn_proj)
    threshold, leak, steepness = map(f32, (threshold, leak, steepness))
    ln2_w, ln2_b, w_fc, b_fc = map(f32, (ln2_w, ln2_b, w_fc, b_fc))
    w_mlp_proj, b_mlp_proj = f32(w_mlp_proj), f32(b_mlp_proj)

    # fold LN affine into the following matmuls (exact in fp32 algebra)
    wa = w_attn * ln1_w[:, None]
    ba = b_attn + ln1_b @ w_attn
    # fold 1/sqrt(D) into the q columns
    wa = wa.copy()
    wa[:, :C] *= 1.0 / np.sqrt(D)
    ba = ba.copy()
    ba[:C] *= 1.0 / np.sqrt(D)
    wf = w_fc * ln2_w[:, None]
    bf = b_fc + ln2_b @ w_fc

    # per-head LIF constants
    st = _softplus(steepness)
    lk = 1.0 / (1.0 + np.exp(-leak))
    th = np.abs(threshold) * 0.1

    wf_b = _bf16(wf)
    wm_b = _bf16(w_mlp_proj)

    in_maps = []
    for c in range(N_CORES):
        b = c // GROUP
        r = c % GROUP
        h0 = r * HL * D  # first local head feature col
        cols = (list(range(h0, h0 + LC))
                + list(range(C + h0, C + h0 + LC))
                + list(range(2 * C + h0, 2 * C + h0 + LC)))
        wqkv_local = _bf16(wa[:, cols])
        bqkv_local = np.ascontiguousarray(ba[cols], dtype=np.float32)
        wproj_local = _bf16(w_attn_proj[h0:h0 + LC, :])
        hsl = slice(r * HL, (r + 1) * HL)
        lif_local = np.stack([
            st[hsl] / 2.0,
            -(st[hsl] * th[hsl]) / 2.0,
            0.5 * (1.0 - lk[hsl]),
            0.5 * (1.0 + lk[hsl]),
        ]).astype(np.float32)
        x_b_core = _bf16(x[b])
        # MLP-phase tokens: RS chunk qs gives rank r rows qs*512+r*128..+128
        x_res_core = np.ascontiguousarray(np.concatenate(
            [x[b][qs * 512 + r * P: qs * 512 + (r + 1) * P] for qs in range(4)]))
        in_maps.append({
            "x_b": x_b_core,
            "x_res": x_res_core,
            "wqkv": wqkv_local,
            "bqkv": bqkv_local,
            "wproj": wproj_local,
            "bproj": b_attn_proj,
            "wfc": wf_b,
            "bfc": bf.astype(np.float32),
            "wmlp": wm_b,
            "bmlp": b_mlp_proj,
            "lif": lif_local,
        })

    global _last_in_maps
    _last_in_maps = in_maps
    nc = _get_nc()
    res = run_bass_kernel_spmd(nc, in_maps, list(range(N_CORES)))

    out = np.empty((B, T, C), np.float32)
    for c in range(N_CORES):
        b = c // GROUP
        r = c % GROUP
        for qs in range(4):
            out[b, qs * 512 + r * P: qs * 512 + (r + 1) * P, :] = \
                res.results[c]["out"][qs * P:(qs + 1) * P]
    return out


# revision 39
# speedup vs baseline: 1.0132x; 1.0132x over previous
"""Trainium2 Bass kernel for nn_EmberBlock (dense transformer block with LIF-gated
attention). 8-core SPMD: head-parallel attention (4 heads/core, one batch per
4-core group) + sequence-parallel MLP after chunked bf16 ReduceScatters.

Attention runs in S^T (key-major) layout: scores are computed transposed so the
gated probabilities feed P·V directly as the moving operand — no per-tile PE
transposes of the attention matrix and no PSUM->SBUF copies for it. Softmax
column sums come from ones-vector matmuls; per-query normalizers are broadcast
back across partitions with rank-1 matmuls; the post-gate renormalizer rides a
ones-column appended to V and is applied while evacuating the PV PSUM.

kernel(**inputs) takes FULL unsharded inputs (as in reference.setup_inputs())
and returns the FULL [B, T, C] output.
"""
import numpy as np
import ml_dtypes

import concourse.bass as bass
import concourse.mybir as mybir
import concourse.tile as tile
from concourse import bacc
from concourse.bass_utils import run_bass_kernel_spmd

F32 = mybir.dt.float32
BF16 = mybir.dt.bfloat16
AF = mybir.ActivationFunctionType
ALU = mybir.AluOpType

# model dims (hardcoded per spec)
B, T, C = 2, 2048, 1024
H, D = 16, 64
FF = 4 * C                    # 4096
N_CORES = 8
GROUP = 4                     # cores per batch
HL = H // GROUP               # 4 local heads
LC = HL * D                   # 256 local head feature cols
EPS_LN = 1e-5
P = 128                       # partitions
NT = T // P                   # 16 token tiles per batch
NQS = 4                       # q-slabs of 512 tokens
MLP_TOK = 512                 # tokens per core in MLP phase (4 strips of 128)

_CACHED_NC = None


def _build():
    nc = bacc.Bacc(None, target_bir_lowering=False, debug=False, num_devices=N_CORES)

    # ---------------- I/O ----------------
    x_b = nc.dram_tensor("x_b", [T, C], BF16, kind="ExternalInput")
    x_res = nc.dram_tensor("x_res", [MLP_TOK, C], F32, kind="ExternalInput")
    wqkv = nc.dram_tensor("wqkv", [C, 3 * LC], BF16, kind="ExternalInput")
    bqkv = nc.dram_tensor("bqkv", [3 * LC], F32, kind="ExternalInput")
    wproj = nc.dram_tensor("wproj", [LC, C], BF16, kind="ExternalInput")
    bproj = nc.dram_tensor("bproj", [C], F32, kind="ExternalInput")
    wfc = nc.dram_tensor("wfc", [C, FF], BF16, kind="ExternalInput")
    bfc = nc.dram_tensor("bfc", [FF], F32, kind="ExternalInput")
    wmlp = nc.dram_tensor("wmlp", [FF, C], BF16, kind="ExternalInput")
    bmlp = nc.dram_tensor("bmlp", [C], F32, kind="ExternalInput")
    lif = nc.dram_tensor("lif", [4, HL], F32, kind="ExternalInput")
    out = nc.dram_tensor("out", [MLP_TOK, C], F32, kind="ExternalOutput")

    # RS bounce buffers (internal DRAM), bf16 to halve collective traffic
    rs_in = nc.dram_tensor("rs_in", [T, C], BF16)
    rs_out = nc.dram_tensor("rs_out", [MLP_TOK, C], BF16)

    # constants embedded in the NEFF
    id_bf = nc.inline_tensor(np.eye(P, dtype=ml_dtypes.bfloat16), name="id_bf")
    # causal mask for a diagonal 128x128 block: mask[r, c] = 1 iff c >= r
    mk = np.triu(np.ones((P, P), np.float32))
    mask_c = nc.inline_tensor(mk.astype(ml_dtypes.bfloat16), name="mask_c")

    replica_groups = [[0, 1, 2, 3], [4, 5, 6, 7]]

    from contextlib import ExitStack
    with tile.TileContext(nc) as tc, ExitStack() as root_ctx:
        attn_ctx = ExitStack()
        consts = root_ctx.enter_context(tc.tile_pool(name="consts", bufs=1))
        zero_c = consts.tile([P, 1], F32)
        nc.vector.memset(zero_c[:], 0.0)
        nc.const_aps.aps[(F32, 0.0)] = zero_c[:]
        eps_c = consts.tile([P, 1], F32)
        nc.vector.memset(eps_c[:], EPS_LN)
        nc.const_aps.aps[(F32, EPS_LN)] = eps_c[:]
        ident = consts.tile([P, P], BF16)
        nc.sync.dma_start(out=ident[:], in_=id_bf[:, :])
        masks = consts.tile([P, P], BF16)
        nc.sync.dma_start(out=masks[:], in_=mask_c[:, :])
        ones_col = consts.tile([P, 1], BF16)
        nc.vector.memset(ones_col[:], 1.0)
        ones_full = consts.tile([P, P], BF16)
        nc.vector.memset(ones_full[:], 1.0)
        ones_row = consts.tile([1, P], BF16)
        nc.vector.memset(ones_row[:], 1.0)
        # per-head LIF constants broadcast to all partitions: [128, 4, HL]
        lif_sb = consts.tile([P, 4, HL], F32)
        nc.sync.dma_start(out=lif_sb[:], in_=lif[None, :, :].to_broadcast((P, 4, HL)))
        # biases in per-partition layout
        bqkv_sb = consts.tile([P, 6], F32)
        nc.sync.dma_start(out=bqkv_sb[:], in_=bqkv.rearrange("(t p) -> p t", p=P))
        bfc_sb = consts.tile([P, FF // P], F32)
        nc.sync.dma_start(out=bfc_sb[:], in_=bfc.rearrange("(t p) -> p t", p=P))
        # attention-projection weights (2 k-tiles), small -> consts
        wproj_sb = consts.tile([P, 2, C], BF16)
        for kt in range(2):
            nc.sync.dma_start(out=wproj_sb[:, kt, :], in_=wproj[kt * P:(kt + 1) * P, :])

        # =========== Phase 1: LN1 over all T tokens + h^T ===========
        stat_pool = root_ctx.enter_context(tc.tile_pool(name="stats", bufs=4))
        mm_psum = root_ctx.enter_context(tc.tile_pool(name="mm_psum", bufs=2, space="PSUM"))
        wfcp1 = root_ctx.enter_context(tc.tile_pool(name="wfcp1", bufs=1))
        wfc_sb1 = wfcp1.tile([P, 8, FF // 2], BF16)
        attn = attn_ctx.enter_context(tc.tile_pool(name="attn", bufs=1))
        p12_ctx = ExitStack()
        p12 = p12_ctx.enter_context(tc.tile_pool(name="p12", bufs=1))
        xio = p12_ctx.enter_context(tc.tile_pool(name="xio", bufs=4))
        tp_ctx = ExitStack()
        tp_psum = tp_ctx.enter_context(tc.tile_pool(name="tp_psum", bufs=4, space="PSUM"))

        hT = p12.tile([P, C // P, T], BF16)       # [128, 8, 2048] feature-major h
        wqkv_sb = p12.tile([P, 8, 3 * LC], BF16)  # 8 k-tiles of wqkv
        for kt in range(8):
            nc.sync.dma_start(out=wqkv_sb[:, kt, :], in_=wqkv[kt * P:(kt + 1) * P, :])

        def layernorm_tile(x_tile, h_out, tag):
            """x_tile [128, C] -> h_out [128, C] bf16 (normalized, no affine)."""
            stats = stat_pool.tile([P, 2, 6], F32, name=f"st_{tag}")
            nc.vector.bn_stats(out=stats[:, 0, :], in_=x_tile[:, 0:512])
            nc.vector.bn_stats(out=stats[:, 1, :], in_=x_tile[:, 512:1024])
            mv = stat_pool.tile([P, 2], F32, name=f"mv_{tag}")
            nc.vector.bn_aggr(out=mv[:], in_=stats[:])
            std = stat_pool.tile([P, 1], F32, name=f"sd_{tag}")
            nc.scalar.activation(std[:], mv[:, 1:2], AF.Sqrt, bias=EPS_LN)
            rstd = stat_pool.tile([P, 1], F32, name=f"rs_{tag}")
            nc.vector.reciprocal(rstd[:], std[:])
            nmr = stat_pool.tile([P, 1], F32, name=f"nm_{tag}")
            nc.vector.tensor_scalar(out=nmr[:], in0=mv[:, 0:1], scalar1=rstd[:],
                                    scalar2=-1.0, op0=ALU.mult, op1=ALU.mult)
            nc.scalar.activation(h_out, x_tile, AF.Identity, bias=nmr[:], scale=rstd[:])

        qkvT = attn.tile([P, 6, T], BF16)   # rows: q(2 tiles) k(2) v(2)

        # Phases 1+2 interleaved per 512-token chunk: LN+transpose of 4 tiles,
        # then that chunk's QKV matmuls (keeps PE busy during DMA/LN latency)
        for ns in range(4):
            for tt in range(4 * ns, 4 * ns + 4):
                x_tile = xio.tile([P, C], BF16, name="x_t", tag="x_t")
                nc.sync.dma_start(out=x_tile[:], in_=x_b[tt * P:(tt + 1) * P, :])
                h_tile = xio.tile([P, C], BF16, name="h_t", tag="h_t")
                layernorm_tile(x_tile[:], h_tile[:], f"ln1_{tt}")
                # transpose h [128, C] -> hT[:, ft, tt*128:...]
                for fg in range(2):  # groups of 4 feature tiles -> one psum bank
                    tp = tp_psum.tile([P, 4, P], BF16, name="htp", tag="htp")
                    for j in range(4):
                        ft = fg * 4 + j
                        nc.tensor.transpose(tp[:, j, :], h_tile[:, ft * P:(ft + 1) * P],
                                            ident[:])
                    if fg == 0:
                        nc.scalar.activation(
                            hT[:, 0:4, tt * P:(tt + 1) * P], tp[:], AF.Identity)
                    else:
                        nc.vector.tensor_copy(
                            hT[:, 4:8, tt * P:(tt + 1) * P], tp[:])
            for mt in range(6):
                ps = mm_psum.tile([P, 512], F32, name="qkv_ps", tag="qkv_ps")
                for kt in range(8):
                    nc.tensor.matmul(ps[:],
                                     wqkv_sb[:, kt, mt * P:(mt + 1) * P],
                                     hT[:, kt, ns * 512:(ns + 1) * 512],
                                     start=(kt == 0), stop=(kt == 7))
                nc.scalar.activation(qkvT[:, mt, ns * 512:(ns + 1) * 512], ps[:],
                                     AF.Identity, bias=bqkv_sb[:, mt:mt + 1])

        # first wfc half preload (x loads are queued; overlaps attention)
        for kt in range(8):
            nc.sync.dma_start(out=wfc_sb1[:, kt, :],
                              in_=wfc[kt * P:(kt + 1) * P, 0:FF // 2])

        # =========== Phase 3: V^T -> V (token-major, +ones col for renorm) ===
        v1 = attn.tile([P, NT, HL, D + 1], BF16)   # [128, 16, 4, 65]
        nc.vector.memset(v1[:, :, :, D:D + 1], 1.0)
        for vt in range(2):
            for tg in range(4):  # 4 token tiles per psum bank
                tp = tp_psum.tile([P, 4, P], BF16, name="vtp", tag="htp")
                for j in range(4):
                    tt = tg * 4 + j
                    nc.tensor.transpose(tp[:, j, :],
                                        qkvT[:, 4 + vt, tt * P:(tt + 1) * P],
                                        ident[:])
                nc.vector.tensor_copy(
                    v1[:, tg * 4:(tg + 1) * 4, 2 * vt:2 * vt + 2, 0:D],
                    tp[:])
        tp_ctx.close()   # release transpose psum banks for attention
        p12_ctx.close()  # hT / wqkv dead after QKV

        # =========== Phase 4: attention, S^T (key-major) layout ===========
        epool = attn_ctx.enter_context(tc.tile_pool(name="epool", bufs=2))
        zpool = attn_ctx.enter_context(tc.tile_pool(name="zpool", bufs=1))
        rowp = attn_ctx.enter_context(tc.tile_pool(name="rowp", bufs=1))
        bcsb = attn_ctx.enter_context(tc.tile_pool(name="bcsb", bufs=2))
        psA = attn_ctx.enter_context(tc.tile_pool(name="psA", bufs=3, space="PSUM"))
        psSE = attn_ctx.enter_context(tc.tile_pool(name="psSE", bufs=2, space="PSUM"))
        psY = attn_ctx.enter_context(tc.tile_pool(name="psY", bufs=1, space="PSUM"))
        ypool = attn_ctx.enter_context(tc.tile_pool(name="ypool", bufs=2))
        o2pool = attn_ctx.enter_context(tc.tile_pool(name="o2pool", bufs=2))

        yTs = {}

        def stageA(qs, h):
            """S^T tiles + exp + column-sum accumulation for head h."""
            nkb, q0 = qs * 4 + 4, qs * 512
            qrow = (h % 2) * D
            qtile = h // 2
            ktile = 2 + h // 2
            eT = epool.tile([P, NT, 512], BF16, name=f"eT{h % 2}",
                            tag=f"eT{h % 2}")
            se = psSE.tile([P, 512], F32, name="se", tag="se")
            q_ap = qkvT[qrow:qrow + D, qtile, q0:q0 + 512]
            pend = []  # (kb, c0, psum) awaiting exp+sum

            def drain():
                kb, c0, ps = pend.pop(0)
                nc.scalar.activation(eT[:, kb, c0:], ps[:, c0:], AF.Exp)
                if kb >= qs * 4:  # diagonal tile: mask the 128-wide block
                    nc.vector.tensor_tensor(out=eT[:, kb, c0:c0 + P],
                                            in0=eT[:, kb, c0:c0 + P],
                                            in1=masks[:], op=ALU.mult)
                nc.tensor.matmul(se[:, c0:], ones_full[:], eT[:, kb, c0:],
                                 start=(kb == 0), stop=(kb == nkb - 1))

            for kb in range(nkb):
                c0 = max(0, kb * P - q0)
                ps = psA.tile([P, 512], F32, name="sps", tag="sps")
                nc.tensor.matmul(
                    ps[:, c0:],
                    qkvT[qrow:qrow + D, ktile, kb * P:(kb + 1) * P],
                    q_ap[:, c0:] if c0 else q_ap,
                    start=True, stop=True)
                pend.append((kb, c0, ps))
                if len(pend) > 2:
                    drain()
            while pend:
                drain()
            state[(qs, h)] = (eT, se)

        def stageB(qs, h):
            """1/se broadcast to all partitions (bf16)."""
            eT, se = state[(qs, h)]
            rse_f = bcsb.tile([P, 512], F32, name=f"rsef{h % 2}",
                              tag=f"rsef{h % 2}")
            nc.vector.reciprocal_approx_fast(rse_f[:], se[:])
            bc = bcsb.tile([P, 512], BF16, name=f"bc{h % 2}", tag=f"bc{h % 2}")
            nc.vector.tensor_copy(bc[:], rse_f[:])
            state[(qs, h)] = (eT, bc)

        def stageCDE(qs, h):
            """Gate, PV, renormalize into yT for head h."""
            nkb, q0 = qs * 4 + 4, qs * 512
            qrow = (h % 2) * D
            eT, bc = state.pop((qs, h))
            ensure_yT(qs)
            yT = yTs[qs]
            zf = zpool.tile([P, NT, 512], BF16, name=f"zf{h % 2}",
                            tag=f"zf{h % 2}")
            for kb in range(nkb):
                c0 = max(0, kb * P - q0)
                # p = e / se  (true softmax prob)
                nc.vector.tensor_tensor(out=zf[:, kb, c0:], in0=eT[:, kb, c0:],
                                        in1=bc[:, c0:], op=ALU.mult)
            # single big instructions over the whole slab (dead regions of
            # diagonal tiles hold garbage; PV skips them via c0 slices)
            # fire-gate: tanh((st/2) p - st th / 2)
            nc.scalar.activation(zf[:, 0:nkb, :], zf[:, 0:nkb, :], AF.Tanh,
                                 bias=lif_sb[:, 1, h:h + 1],
                                 scale=lif_sb[:, 0, h:h + 1])
            # w = c1 tanh + c0 ; m = w * e (renormalized later)
            nc.vector.tensor_scalar(out=zf[:, 0:nkb, :], in0=zf[:, 0:nkb, :],
                                    scalar1=lif_sb[:, 2, h:h + 1],
                                    scalar2=lif_sb[:, 3, h:h + 1],
                                    op0=ALU.mult, op1=ALU.add)
            nc.vector.tensor_tensor(out=zf[:, 0:nkb, :], in0=zf[:, 0:nkb, :],
                                    in1=eT[:, 0:nkb, :], op=ALU.mult)
            # PV with appended ones row: yps[0:64] = y^T, yps[64] = sum m
            yps = psY.tile([D + 1, 512], F32, name="yps", tag="yps")
            for kb in range(nkb):
                c0 = max(0, kb * P - q0)
                nc.tensor.matmul(yps[:, c0:], v1[:, kb, h, :],
                                 zf[:, kb, c0:],
                                 start=(kb == 0), stop=(kb == nkb - 1))
            # renormalize: yT = yps[0:64] / sum
            sm_s = rowp.tile([1, 512], F32, name="sms", tag="sms")
            nc.scalar.activation(sm_s[:], yps[D:D + 1, :], AF.Identity)
            rsm = rowp.tile([1, 512], F32, name="rsm", tag="rsm")
            nc.vector.reciprocal_approx_fast(rsm[:], sm_s[:])
            rsm_b = rowp.tile([1, 512], BF16, name="rsmb", tag="rsmb")
            nc.vector.tensor_copy(rsm_b[:], rsm[:])
            bc2_ps = mm_psum.tile([P, 512], F32, name="bc2", tag="qkv_ps")
            nc.tensor.matmul(bc2_ps[0:D, :], ones_row[:, 0:D], rsm_b[:],
                             start=True, stop=True)
            bc2 = bcsb.tile([P, 512], BF16, name=f"bc2{h % 2}",
                            tag=f"bc{h % 2}")
            nc.scalar.activation(bc2[0:D, :], bc2_ps[0:D, :], AF.Identity)
            nc.vector.tensor_tensor(out=yT[h // 2][qrow:qrow + D, :],
                                    in0=yps[0:D, :], in1=bc2[0:D, :],
                                    op=ALU.mult)

        def proj_rs(qs):
            """Attention projection + ReduceScatter chunk for slab qs."""
            q0 = qs * 512
            yT = yTs.pop(qs)
            for mt in range(4):
                o2 = o2pool.tile([P, C], BF16, name="o2", tag="o2")
                for ns in range(2):
                    ps = mm_psum.tile([P, 512], F32, name="o2_ps", tag="qkv_ps")
                    for kt in range(2):
                        nc.tensor.matmul(ps[:],
                                         yT[kt][:, mt * P:(mt + 1) * P],
                                         wproj_sb[:, kt, ns * 512:(ns + 1) * 512],
                                         start=(kt == 0), stop=(kt == 1))
                    nc.vector.tensor_copy(o2[:, ns * 512:(ns + 1) * 512], ps[:])
                nc.sync.dma_start(
                    out=rs_in[q0 + mt * P: q0 + (mt + 1) * P, :],
                    in_=o2[:])
            # chunked ReduceScatter: this slab's 512 rows -> 128 local rows
            nc.gpsimd.collective_compute(
                "ReduceScatter", ALU.add, replica_groups=replica_groups,
                ins=[rs_in[q0:q0 + 512, :]],
                outs=[rs_out[qs * P:(qs + 1) * P, :]])

        # continuous 16-step (slab, head) pipeline — no drain at slab bounds
        state = {}
        items = [(qs, h) for qs in reversed(range(NQS)) for h in range(HL)]
        def ensure_yT(qs):
            if qs not in yTs:
                yTs[qs] = [ypool.tile([P, 512], BF16, name=f"yT{i}_{qs}",
                                      tag=f"yT{i}") for i in range(2)]
        stageA(*items[0])
        for i in range(1, len(items)):
            stageB(*items[i - 1])
            stageA(*items[i])
            if i >= 3:
                stageCDE(*items[i - 3])
                if items[i - 3][1] == HL - 1:
                    proj_rs(items[i - 3][0])
        stageB(*items[-1])
        for i in (-3, -2, -1):
            stageCDE(*items[i])
            if items[i][1] == HL - 1:
                proj_rs(items[i][0])

        # release attention pools
        attn_ctx.close()

        # =========== Phase 5: MLP on 512 local tokens ===========
        wfcp2 = root_ctx.enter_context(tc.tile_pool(name="wfcp2", bufs=1))
        wfc_sb2 = wfcp2.tile([P, 8, FF // 2], BF16)
        for kt in range(8):
            nc.sync.dma_start(out=wfc_sb2[:, kt, :],
                              in_=wfc[kt * P:(kt + 1) * P, FF // 2:])
        mlp = root_ctx.enter_context(tc.tile_pool(name="mlp", bufs=1))
        wstream = root_ctx.enter_context(tc.tile_pool(name="wstream", bufs=2))
        tp2_psum = root_ctx.enter_context(tc.tile_pool(name="tp2", bufs=2,
                                                       space="PSUM"))
        o3_psum = root_ctx.enter_context(tc.tile_pool(name="o3p", bufs=2,
                                                      space="PSUM"))

        # free-dim biases broadcast across partitions
        bproj_sb = mlp.tile([P, C], BF16)
        nc.gpsimd.dma_start(out=bproj_sb[:], in_=bproj[None, :].to_broadcast((P, C)))
        bmlp_sb = mlp.tile([P, C], BF16)
        nc.gpsimd.dma_start(out=bmlp_sb[:], in_=bmlp[None, :].to_broadcast((P, C)))
        h2T = mlp.tile([P, 8, MLP_TOK], BF16)
        aT = mlp.tile([P, FF // P, MLP_TOK], BF16)
        x1_t = {}
        for half in (1, 0):
            for j in (2 * half, 2 * half + 1):
                rs_sb = mlp.tile([P, C], BF16, name=f"rs_sb{j}", tag="rs_sb")
                nc.sync.dma_start(out=rs_sb[:], in_=rs_out[j * P:(j + 1) * P, :])
                xr = mlp.tile([P, C], F32, name=f"xr{j}", tag="xr")
                nc.sync.dma_start(out=xr[:], in_=x_res[j * P:(j + 1) * P, :])
                x1 = mlp.tile([P, C], F32, name=f"x1_{j}")
                nc.vector.tensor_tensor(out=x1[:], in0=xr[:], in1=rs_sb[:],
                                        op=ALU.add)
                nc.vector.tensor_tensor(out=x1[:], in0=x1[:], in1=bproj_sb[:],
                                        op=ALU.add)
                x1_t[j] = x1
                h2 = mlp.tile([P, C], BF16, name=f"h2_{j}", tag="h2")
                layernorm_tile(x1[:], h2[:], f"ln2_{j}")
                for fg in range(2):
                    tp = tp2_psum.tile([P, 4, P], BF16, name="h2tp", tag="h2tp")
                    for k in range(4):
                        ft = fg * 4 + k
                        nc.tensor.transpose(tp[:, k, :], h2[:, ft * P:(ft + 1) * P],
                                            ident[:])
                    nc.vector.tensor_copy(
                        h2T[:, fg * 4:(fg + 1) * 4, j * P:(j + 1) * P], tp[:])
            # FC + gelu on this 256-token half
            for mt in range(FF // P):
                wsb = wfc_sb1 if mt < 16 else wfc_sb2
                mt0 = mt if mt < 16 else mt - 16
                ps = o3_psum.tile([P, 256], F32, name="fc_ps", tag="o3_ps")
                for kt in range(8):
                    nc.tensor.matmul(ps[:], wsb[:, kt, mt0 * P:(mt0 + 1) * P],
                                     h2T[:, kt, half * 256:(half + 1) * 256],
                                     start=(kt == 0), stop=(kt == 7))
                nc.scalar.activation(aT[:, mt, half * 256:(half + 1) * 256], ps[:],
                                     AF.Gelu, bias=bfc_sb[:, mt:mt + 1])

        # MLP proj + residual accumulated in place into x1
        for ns in range(4):
            wm_q = wstream.tile([P, FF // P, 256], BF16, name="wm_q", tag="wm_q")
            for kt in range(FF // P):
                nc.sync.dma_start(out=wm_q[:, kt, :],
                                  in_=wmlp[kt * P:(kt + 1) * P,
                                           ns * 256:(ns + 1) * 256])
            for j in range(4):
                ps = o3_psum.tile([P, 256], F32, name="o3_ps", tag="o3_ps")
                for kt in range(FF // P):
                    nc.tensor.matmul(ps[:], aT[:, kt, j * P:(j + 1) * P],
                                     wm_q[:, kt, :],
                                     start=(kt == 0), stop=(kt == FF // P - 1))
                nc.vector.tensor_tensor(out=x1_t[j][:, ns * 256:(ns + 1) * 256],
                                        in0=x1_t[j][:, ns * 256:(ns + 1) * 256],
                                        in1=ps[:], op=ALU.add)
        for j in range(4):
            nc.vector.tensor_tensor(out=x1_t[j][:], in0=x1_t[j][:],
                                    in1=bmlp_sb[:], op=ALU.add)
            nc.sync.dma_start(out=out[j * P:(j + 1) * P, :], in_=x1_t[j][:])

    nc.compile()
    return nc


def _get_nc():
    global _CACHED_NC
    if _CACHED_NC is None:
        _CACHED_NC = _build()
    return _CACHED_NC


def _softplus(x):
    return np.log1p(np.exp(-np.abs(x))) + np.maximum(x, 0.0)


def _bf16(x):
    return np.ascontiguousarray(x.astype(ml_dtypes.bfloat16))


def kernel(x, ln1_w, ln1_b, w_attn, b_attn, w_attn_proj, b_attn_proj,
           threshold, leak, steepness, ln2_w, ln2_b,
           w_fc, b_fc, w_mlp_proj, b_mlp_proj):
    x = np.asarray(x, np.float32)
    f32 = lambda a: np.asarray(a, np.float32)
    ln1_w, ln1_b, w_attn, b_attn = map(f32, (ln1_w, ln1_b, w_attn, b_attn))
    w_attn_proj, b_attn_proj = f32(w_attn_proj), f32(b_attn_proj)
    threshold, leak, steepness = map(f32, (threshold, leak, steepness))
    ln2_w, ln2_b, w_fc, b_fc = map(f32, (ln2_w, ln2_b, w_fc, b_fc))
    w_mlp_proj, b_mlp_proj = f32(w_mlp_proj), f32(b_mlp_proj)

    # fold LN affine into the following matmuls (exact in fp32 algebra)
    wa = w_attn * ln1_w[:, None]
    ba = b_attn + ln1_b @ w_attn
    # fold 1/sqrt(D) into the q columns
    wa = wa.copy()
    wa[:, :C] *= 1.0 / np.sqrt(D)
    ba = ba.copy()
    ba[:C] *= 1.0 / np.sqrt(D)
    wf = w_fc * ln2_w[:, None]
    bf = b_fc + ln2_b @ w_fc

    # per-head LIF constants
    st = _softplus(steepness)
    lk = 1.0 / (1.0 + np.exp(-leak))
    th = np.abs(threshold) * 0.1

    wf_b = _bf16(wf)
    wm_b = _bf16(w_mlp_proj)

    in_maps = []
    for c in range(N_CORES):
        b = c // GROUP
        r = c % GROUP
        h0 = r * HL * D  # first local head feature col
        cols = (list(range(h0, h0 + LC))
                + list(range(C + h0, C + h0 + LC))
                + list(range(2 * C + h0, 2 * C + h0 + LC)))
        wqkv_local = _bf16(wa[:, cols])
        bqkv_local = np.ascontiguousarray(ba[cols], dtype=np.float32)
        wproj_local = _bf16(w_attn_proj[h0:h0 + LC, :])
        hsl = slice(r * HL, (r + 1) * HL)
        lif_local = np.stack([
            st[hsl] / 2.0,
            -(st[hsl] * th[hsl]) / 2.0,
            0.5 * (1.0 - lk[hsl]),
            0.5 * (1.0 + lk[hsl]),
        ]).astype(np.float32)
        x_b_core = _bf16(x[b])
        # MLP-phase tokens: RS chunk qs gives rank r rows qs*512+r*128..+128
        x_res_core = np.ascontiguousarray(np.concatenate(
            [x[b][qs * 512 + r * P: qs * 512 + (r + 1) * P] for qs in range(4)]))
        in_maps.append({
            "x_b": x_b_core,
            "x_res": x_res_core,
            "wqkv": wqkv_local,
            "bqkv": bqkv_local,
            "wproj": wproj_local,
            "bproj": b_attn_proj,
            "wfc": wf_b,
            "bfc": bf.astype(np.float32),
            "wmlp": wm_b,
            "bmlp": b_mlp_proj,
            "lif": lif_local,
        })

    global _last_in_maps
    _last_in_maps = in_maps
    nc = _get_nc()
    res = run_bass_kernel_spmd(nc, in_maps, list(range(N_CORES)))

    out = np.empty((B, T, C), np.float32)
    for c in range(N_CORES):
        b = c // GROUP
        r = c % GROUP
        for qs in range(4):
            out[b, qs * 512 + r * P: qs * 512 + (r + 1) * P, :] = \
                res.results[c]["out"][qs * P:(qs + 1) * P]
    return out


# revision 40
# speedup vs baseline: 1.0270x; 1.0136x over previous
"""Trainium2 Bass kernel for nn_EmberBlock (dense transformer block with LIF-gated
attention). 8-core SPMD: head-parallel attention (4 heads/core, one batch per
4-core group) + sequence-parallel MLP after chunked bf16 ReduceScatters.

Attention runs in S^T (key-major) layout: scores are computed transposed so the
gated probabilities feed P·V directly as the moving operand — no per-tile PE
transposes of the attention matrix and no PSUM->SBUF copies for it. Softmax
column sums come from ones-vector matmuls; per-query normalizers are broadcast
back across partitions with rank-1 matmuls; the post-gate renormalizer rides a
ones-column appended to V and is applied while evacuating the PV PSUM.

kernel(**inputs) takes FULL unsharded inputs (as in reference.setup_inputs())
and returns the FULL [B, T, C] output.
"""
import numpy as np
import ml_dtypes

import concourse.bass as bass
import concourse.mybir as mybir
import concourse.tile as tile
from concourse import bacc
from concourse.bass_utils import run_bass_kernel_spmd

F32 = mybir.dt.float32
BF16 = mybir.dt.bfloat16
AF = mybir.ActivationFunctionType
ALU = mybir.AluOpType

# model dims (hardcoded per spec)
B, T, C = 2, 2048, 1024
H, D = 16, 64
FF = 4 * C                    # 4096
N_CORES = 8
GROUP = 4                     # cores per batch
HL = H // GROUP               # 4 local heads
LC = HL * D                   # 256 local head feature cols
EPS_LN = 1e-5
P = 128                       # partitions
NT = T // P                   # 16 token tiles per batch
NQS = 4                       # q-slabs of 512 tokens
MLP_TOK = 512                 # tokens per core in MLP phase (4 strips of 128)

_CACHED_NC = None


def _build():
    nc = bacc.Bacc(None, target_bir_lowering=False, debug=False, num_devices=N_CORES)

    # ---------------- I/O ----------------
    x_b = nc.dram_tensor("x_b", [T, C], BF16, kind="ExternalInput")
    x_res = nc.dram_tensor("x_res", [MLP_TOK, C], F32, kind="ExternalInput")
    wqkv = nc.dram_tensor("wqkv", [C, 3 * LC], BF16, kind="ExternalInput")
    bqkv = nc.dram_tensor("bqkv", [3 * LC], F32, kind="ExternalInput")
    wproj = nc.dram_tensor("wproj", [LC, C], BF16, kind="ExternalInput")
    bproj = nc.dram_tensor("bproj", [C], F32, kind="ExternalInput")
    wfc = nc.dram_tensor("wfc", [C, FF], BF16, kind="ExternalInput")
    bfc = nc.dram_tensor("bfc", [FF], F32, kind="ExternalInput")
    wmlp = nc.dram_tensor("wmlp", [FF, C], BF16, kind="ExternalInput")
    bmlp = nc.dram_tensor("bmlp", [C], F32, kind="ExternalInput")
    lif = nc.dram_tensor("lif", [4, HL], F32, kind="ExternalInput")
    out = nc.dram_tensor("out", [MLP_TOK, C], F32, kind="ExternalOutput")

    # RS bounce buffers (internal DRAM), bf16 to halve collective traffic
    rs_in = nc.dram_tensor("rs_in", [T, C], BF16)
    rs_out = nc.dram_tensor("rs_out", [MLP_TOK, C], BF16)

    # constants embedded in the NEFF
    id_bf = nc.inline_tensor(np.eye(P, dtype=ml_dtypes.bfloat16), name="id_bf")
    # causal mask for a diagonal 128x128 block: mask[r, c] = 1 iff c >= r
    mk = np.triu(np.ones((P, P), np.float32))
    mask_c = nc.inline_tensor(mk.astype(ml_dtypes.bfloat16), name="mask_c")

    replica_groups = [[0, 1, 2, 3], [4, 5, 6, 7]]

    from contextlib import ExitStack
    with tile.TileContext(nc) as tc, ExitStack() as root_ctx:
        attn_ctx = ExitStack()
        consts = root_ctx.enter_context(tc.tile_pool(name="consts", bufs=1))
        zero_c = consts.tile([P, 1], F32)
        nc.vector.memset(zero_c[:], 0.0)
        nc.const_aps.aps[(F32, 0.0)] = zero_c[:]
        eps_c = consts.tile([P, 1], F32)
        nc.vector.memset(eps_c[:], EPS_LN)
        nc.const_aps.aps[(F32, EPS_LN)] = eps_c[:]
        ident = consts.tile([P, P], BF16)
        nc.sync.dma_start(out=ident[:], in_=id_bf[:, :])
        masks = consts.tile([P, P], BF16)
        nc.sync.dma_start(out=masks[:], in_=mask_c[:, :])
        ones_col = consts.tile([P, 1], BF16)
        nc.vector.memset(ones_col[:], 1.0)
        ones_full = consts.tile([P, P], BF16)
        nc.vector.memset(ones_full[:], 1.0)
        ones_row = consts.tile([1, P], BF16)
        nc.vector.memset(ones_row[:], 1.0)
        # per-head LIF constants broadcast to all partitions: [128, 4, HL]
        lif_sb = consts.tile([P, 4, HL], F32)
        nc.sync.dma_start(out=lif_sb[:], in_=lif[None, :, :].to_broadcast((P, 4, HL)))
        # biases in per-partition layout
        bqkv_sb = consts.tile([P, 6], F32)
        nc.sync.dma_start(out=bqkv_sb[:], in_=bqkv.rearrange("(t p) -> p t", p=P))
        bfc_sb = consts.tile([P, FF // P], F32)
        nc.sync.dma_start(out=bfc_sb[:], in_=bfc.rearrange("(t p) -> p t", p=P))
        # attention-projection weights (2 k-tiles), small -> consts
        wproj_sb = consts.tile([P, 2, C], BF16)
        for kt in range(2):
            nc.sync.dma_start(out=wproj_sb[:, kt, :], in_=wproj[kt * P:(kt + 1) * P, :])

        # =========== Phase 1: LN1 over all T tokens + h^T ===========
        stat_pool = root_ctx.enter_context(tc.tile_pool(name="stats", bufs=4))
        mm_psum = root_ctx.enter_context(tc.tile_pool(name="mm_psum", bufs=2, space="PSUM"))
        wfcp1 = root_ctx.enter_context(tc.tile_pool(name="wfcp1", bufs=1))
        wfc_sb1 = wfcp1.tile([P, 8, FF // 2], BF16)
        attn = attn_ctx.enter_context(tc.tile_pool(name="attn", bufs=1))
        p12_ctx = ExitStack()
        p12 = p12_ctx.enter_context(tc.tile_pool(name="p12", bufs=1))
        xio = p12_ctx.enter_context(tc.tile_pool(name="xio", bufs=4))
        tp_ctx = ExitStack()
        tp_psum = tp_ctx.enter_context(tc.tile_pool(name="tp_psum", bufs=4, space="PSUM"))

        hT = p12.tile([P, C // P, T], BF16)       # [128, 8, 2048] feature-major h
        wqkv_sb = p12.tile([P, 8, 3 * LC], BF16)  # 8 k-tiles of wqkv
        for kt in range(8):
            nc.sync.dma_start(out=wqkv_sb[:, kt, :], in_=wqkv[kt * P:(kt + 1) * P, :])

        def layernorm_tile(x_tile, h_out, tag):
            """x_tile [128, C] -> h_out [128, C] bf16 (normalized, no affine)."""
            stats = stat_pool.tile([P, 2, 6], F32, name=f"st_{tag}")
            nc.vector.bn_stats(out=stats[:, 0, :], in_=x_tile[:, 0:512])
            nc.vector.bn_stats(out=stats[:, 1, :], in_=x_tile[:, 512:1024])
            mv = stat_pool.tile([P, 2], F32, name=f"mv_{tag}")
            nc.vector.bn_aggr(out=mv[:], in_=stats[:])
            std = stat_pool.tile([P, 1], F32, name=f"sd_{tag}")
            nc.scalar.activation(std[:], mv[:, 1:2], AF.Sqrt, bias=EPS_LN)
            rstd = stat_pool.tile([P, 1], F32, name=f"rs_{tag}")
            nc.vector.reciprocal(rstd[:], std[:])
            nmr = stat_pool.tile([P, 1], F32, name=f"nm_{tag}")
            nc.vector.tensor_scalar(out=nmr[:], in0=mv[:, 0:1], scalar1=rstd[:],
                                    scalar2=-1.0, op0=ALU.mult, op1=ALU.mult)
            nc.scalar.activation(h_out, x_tile, AF.Identity, bias=nmr[:], scale=rstd[:])

        qkvT = attn.tile([P, 6, T], BF16)   # rows: q(2 tiles) k(2) v(2)

        # Phases 1+2 interleaved per 512-token chunk: LN+transpose of 4 tiles,
        # then that chunk's QKV matmuls (keeps PE busy during DMA/LN latency)
        for ns in range(4):
            for tt in range(4 * ns, 4 * ns + 4):
                x_tile = xio.tile([P, C], BF16, name="x_t", tag="x_t")
                nc.sync.dma_start(out=x_tile[:], in_=x_b[tt * P:(tt + 1) * P, :])
                h_tile = xio.tile([P, C], BF16, name="h_t", tag="h_t")
                layernorm_tile(x_tile[:], h_tile[:], f"ln1_{tt}")
                # transpose h [128, C] -> hT[:, ft, tt*128:...]
                for fg in range(2):  # groups of 4 feature tiles -> one psum bank
                    tp = tp_psum.tile([P, 4, P], BF16, name="htp", tag="htp")
                    for j in range(4):
                        ft = fg * 4 + j
                        nc.tensor.transpose(tp[:, j, :], h_tile[:, ft * P:(ft + 1) * P],
                                            ident[:])
                    if fg == 0:
                        nc.scalar.activation(
                            hT[:, 0:4, tt * P:(tt + 1) * P], tp[:], AF.Identity)
                    else:
                        nc.vector.tensor_copy(
                            hT[:, 4:8, tt * P:(tt + 1) * P], tp[:])
            for mt in range(6):
                ps = mm_psum.tile([P, 512], F32, name="qkv_ps", tag="qkv_ps")
                for kt in range(8):
                    nc.tensor.matmul(ps[:],
                                     wqkv_sb[:, kt, mt * P:(mt + 1) * P],
                                     hT[:, kt, ns * 512:(ns + 1) * 512],
                                     start=(kt == 0), stop=(kt == 7))
                nc.scalar.activation(qkvT[:, mt, ns * 512:(ns + 1) * 512], ps[:],
                                     AF.Identity, bias=bqkv_sb[:, mt:mt + 1])

        # first wfc half preload (x loads are queued; overlaps attention)
        for kt in range(8):
            nc.sync.dma_start(out=wfc_sb1[:, kt, :],
                              in_=wfc[kt * P:(kt + 1) * P, 0:FF // 2])

        # =========== Phase 3: V^T -> V (token-major, +ones col for renorm) ===
        v1 = attn.tile([P, NT, HL, D + 1], BF16)   # [128, 16, 4, 65]
        nc.vector.memset(v1[:, :, :, D:D + 1], 1.0)
        for vt in range(2):
            for tg in range(4):  # 4 token tiles per psum bank
                tp = tp_psum.tile([P, 4, P], BF16, name="vtp", tag="htp")
                for j in range(4):
                    tt = tg * 4 + j
                    nc.tensor.transpose(tp[:, j, :],
                                        qkvT[:, 4 + vt, tt * P:(tt + 1) * P],
                                        ident[:])
                nc.vector.tensor_copy(
                    v1[:, tg * 4:(tg + 1) * 4, 2 * vt:2 * vt + 2, 0:D],
                    tp[:])
        tp_ctx.close()   # release transpose psum banks for attention
        p12_ctx.close()  # hT / wqkv dead after QKV

        # =========== Phase 4: attention, S^T (key-major) layout ===========
        epool = attn_ctx.enter_context(tc.tile_pool(name="epool", bufs=2))
        zpool = attn_ctx.enter_context(tc.tile_pool(name="zpool", bufs=1))
        rowp = attn_ctx.enter_context(tc.tile_pool(name="rowp", bufs=1))
        bcsb = attn_ctx.enter_context(tc.tile_pool(name="bcsb", bufs=2))
        psA = attn_ctx.enter_context(tc.tile_pool(name="psA", bufs=3, space="PSUM"))
        psSE = attn_ctx.enter_context(tc.tile_pool(name="psSE", bufs=2, space="PSUM"))
        psY = attn_ctx.enter_context(tc.tile_pool(name="psY", bufs=1, space="PSUM"))
        ypool = attn_ctx.enter_context(tc.tile_pool(name="ypool", bufs=2))
        o2pool = attn_ctx.enter_context(tc.tile_pool(name="o2pool", bufs=2))

        yTs = {}

        def stageA2(qs, hp):
            """S^T + exp + column sums for the head pair (2*hp, 2*hp+1).

            The two heads' K=64 S-matmuls are interleaved back-to-back; they
            target PE row-groups 0-63 / 64-127 and run concurrently.
            """
            nkb, q0 = qs * 4 + 4, qs * 512
            qtile = hp
            ktile = 2 + hp
            eTs, ses = [], []
            for sub in range(2):
                eTs.append(epool.tile([P, NT, 512], BF16, name=f"eT{sub}",
                                      tag=f"eT{sub}"))
                ses.append(psSE.tile([P, 512], F32, name="se", tag="se"))
            q_aps = [qkvT[sub * D:(sub + 1) * D, qtile, q0:q0 + 512]
                     for sub in range(2)]
            pend = []

            def drain():
                kb, c0, pss = pend.pop(0)
                for sub in range(2):
                    nc.scalar.activation(eTs[sub][:, kb, c0:], pss[sub][:, c0:],
                                         AF.Exp)
                    if kb >= qs * 4:
                        nc.vector.tensor_tensor(out=eTs[sub][:, kb, c0:c0 + P],
                                                in0=eTs[sub][:, kb, c0:c0 + P],
                                                in1=masks[:], op=ALU.mult)
                    nc.tensor.matmul(ses[sub][:, c0:], ones_full[:],
                                     eTs[sub][:, kb, c0:],
                                     start=(kb == 0), stop=(kb == nkb - 1))

            for kb in range(nkb):
                c0 = max(0, kb * P - q0)
                pss = []
                for sub in range(2):
                    ps = psA.tile([P, 512], F32, name="sps", tag="sps")
                    nc.tensor.matmul(
                        ps[:, c0:],
                        qkvT[sub * D:(sub + 1) * D, ktile, kb * P:(kb + 1) * P],
                        q_aps[sub][:, c0:] if c0 else q_aps[sub],
                        start=True, stop=True)
                    pss.append(ps)
                pend.append((kb, c0, pss))
                if len(pend) > 1:
                    drain()
            while pend:
                drain()
            for sub in range(2):
                state[(qs, 2 * hp + sub)] = (eTs[sub], ses[sub])

        def stageA(qs, h):
            """S^T tiles + exp + column-sum accumulation for head h."""
            nkb, q0 = qs * 4 + 4, qs * 512
            qrow = (h % 2) * D
            qtile = h // 2
            ktile = 2 + h // 2
            eT = epool.tile([P, NT, 512], BF16, name=f"eT{h % 2}",
                            tag=f"eT{h % 2}")
            se = psSE.tile([P, 512], F32, name="se", tag="se")
            q_ap = qkvT[qrow:qrow + D, qtile, q0:q0 + 512]
            pend = []  # (kb, c0, psum) awaiting exp+sum

            def drain():
                kb, c0, ps = pend.pop(0)
                nc.scalar.activation(eT[:, kb, c0:], ps[:, c0:], AF.Exp)
                if kb >= qs * 4:  # diagonal tile: mask the 128-wide block
                    nc.vector.tensor_tensor(out=eT[:, kb, c0:c0 + P],
                                            in0=eT[:, kb, c0:c0 + P],
                                            in1=masks[:], op=ALU.mult)
                nc.tensor.matmul(se[:, c0:], ones_full[:], eT[:, kb, c0:],
                                 start=(kb == 0), stop=(kb == nkb - 1))

            for kb in range(nkb):
                c0 = max(0, kb * P - q0)
                ps = psA.tile([P, 512], F32, name="sps", tag="sps")
                nc.tensor.matmul(
                    ps[:, c0:],
                    qkvT[qrow:qrow + D, ktile, kb * P:(kb + 1) * P],
                    q_ap[:, c0:] if c0 else q_ap,
                    start=True, stop=True)
                pend.append((kb, c0, ps))
                if len(pend) > 2:
                    drain()
            while pend:
                drain()
            state[(qs, h)] = (eT, se)

        def stageB(qs, h):
            """1/se broadcast to all partitions (bf16)."""
            eT, se = state[(qs, h)]
            rse_f = bcsb.tile([P, 512], F32, name=f"rsef{h % 2}",
                              tag=f"rsef{h % 2}")
            nc.vector.reciprocal_approx_fast(rse_f[:], se[:])
            bc = bcsb.tile([P, 512], BF16, name=f"bc{h % 2}", tag=f"bc{h % 2}")
            nc.vector.tensor_copy(bc[:], rse_f[:])
            state[(qs, h)] = (eT, bc)

        def stageCDE(qs, h):
            """Gate, PV, renormalize into yT for head h."""
            nkb, q0 = qs * 4 + 4, qs * 512
            qrow = (h % 2) * D
            eT, bc = state.pop((qs, h))
            ensure_yT(qs)
            yT = yTs[qs]
            zf = zpool.tile([P, NT, 512], BF16, name=f"zf{h % 2}",
                            tag=f"zf{h % 2}")
            for kb in range(nkb):
                c0 = max(0, kb * P - q0)
                # p = e / se  (true softmax prob)
                nc.vector.tensor_tensor(out=zf[:, kb, c0:], in0=eT[:, kb, c0:],
                                        in1=bc[:, c0:], op=ALU.mult)
            # single big instructions over the whole slab (dead regions of
            # diagonal tiles hold garbage; PV skips them via c0 slices)
            # fire-gate: tanh((st/2) p - st th / 2)
            nc.scalar.activation(zf[:, 0:nkb, :], zf[:, 0:nkb, :], AF.Tanh,
                                 bias=lif_sb[:, 1, h:h + 1],
                                 scale=lif_sb[:, 0, h:h + 1])
            # w = c1 tanh + c0 ; m = w * e (renormalized later)
            nc.vector.tensor_scalar(out=zf[:, 0:nkb, :], in0=zf[:, 0:nkb, :],
                                    scalar1=lif_sb[:, 2, h:h + 1],
                                    scalar2=lif_sb[:, 3, h:h + 1],
                                    op0=ALU.mult, op1=ALU.add)
            nc.vector.tensor_tensor(out=zf[:, 0:nkb, :], in0=zf[:, 0:nkb, :],
                                    in1=eT[:, 0:nkb, :], op=ALU.mult)
            # PV with appended ones row: yps[0:64] = y^T, yps[64] = sum m
            yps = psY.tile([D + 1, 512], F32, name="yps", tag="yps")
            for kb in range(nkb):
                c0 = max(0, kb * P - q0)
                nc.tensor.matmul(yps[:, c0:], v1[:, kb, h, :],
                                 zf[:, kb, c0:],
                                 start=(kb == 0), stop=(kb == nkb - 1))
            # renormalize: yT = yps[0:64] / sum
            sm_s = rowp.tile([1, 512], F32, name="sms", tag="sms")
            nc.scalar.activation(sm_s[:], yps[D:D + 1, :], AF.Identity)
            rsm = rowp.tile([1, 512], F32, name="rsm", tag="rsm")
            nc.vector.reciprocal_approx_fast(rsm[:], sm_s[:])
            rsm_b = rowp.tile([1, 512], BF16, name="rsmb", tag="rsmb")
            nc.vector.tensor_copy(rsm_b[:], rsm[:])
            bc2_ps = mm_psum.tile([P, 512], F32, name="bc2", tag="qkv_ps")
            nc.tensor.matmul(bc2_ps[0:D, :], ones_row[:, 0:D], rsm_b[:],
                             start=True, stop=True)
            bc2 = bcsb.tile([P, 512], BF16, name=f"bc2{h % 2}",
                            tag=f"bc{h % 2}")
            nc.scalar.activation(bc2[0:D, :], bc2_ps[0:D, :], AF.Identity)
            nc.vector.tensor_tensor(out=yT[h // 2][qrow:qrow + D, :],
                                    in0=yps[0:D, :], in1=bc2[0:D, :],
                                    op=ALU.mult)

        def proj_rs(qs):
            """Attention projection + ReduceScatter chunk for slab qs."""
            q0 = qs * 512
            yT = yTs.pop(qs)
            for mt in range(4):
                o2 = o2pool.tile([P, C], BF16, name="o2", tag="o2")
                for ns in range(2):
                    ps = mm_psum.tile([P, 512], F32, name="o2_ps", tag="qkv_ps")
                    for kt in range(2):
                        nc.tensor.matmul(ps[:],
                                         yT[kt][:, mt * P:(mt + 1) * P],
                                         wproj_sb[:, kt, ns * 512:(ns + 1) * 512],
                                         start=(kt == 0), stop=(kt == 1))
                    nc.vector.tensor_copy(o2[:, ns * 512:(ns + 1) * 512], ps[:])
                nc.sync.dma_start(
                    out=rs_in[q0 + mt * P: q0 + (mt + 1) * P, :],
                    in_=o2[:])
            # chunked ReduceScatter: this slab's 512 rows -> 128 local rows
            nc.gpsimd.collective_compute(
                "ReduceScatter", ALU.add, replica_groups=replica_groups,
                ins=[rs_in[q0:q0 + 512, :]],
                outs=[rs_out[qs * P:(qs + 1) * P, :]])

        # continuous 8-step (slab, head-pair) pipeline — no drain at slab bounds
        state = {}
        items = [(qs, hp) for qs in reversed(range(NQS)) for hp in range(2)]
        def ensure_yT(qs):
            if qs not in yTs:
                yTs[qs] = [ypool.tile([P, 512], BF16, name=f"yT{i}_{qs}",
                                      tag=f"yT{i}") for i in range(2)]
        def stageB2(qs, hp):
            stageB(qs, 2 * hp)
            stageB(qs, 2 * hp + 1)

        def stageCDE2(qs, hp):
            stageCDE(qs, 2 * hp)
            stageCDE(qs, 2 * hp + 1)
            if hp == 1:
                proj_rs(qs)

        stageA2(*items[0])
        for i in range(1, len(items)):
            stageB2(*items[i - 1])
            stageA2(*items[i])
            stageCDE2(*items[i - 1])
        stageB2(*items[-1])
        stageCDE2(*items[-1])

        # release attention pools
        attn_ctx.close()

        # =========== Phase 5: MLP on 512 local tokens ===========
        wfcp2 = root_ctx.enter_context(tc.tile_pool(name="wfcp2", bufs=1))
        wfc_sb2 = wfcp2.tile([P, 8, FF // 2], BF16)
        for kt in range(8):
            nc.sync.dma_start(out=wfc_sb2[:, kt, :],
                              in_=wfc[kt * P:(kt + 1) * P, FF // 2:])
        mlp = root_ctx.enter_context(tc.tile_pool(name="mlp", bufs=1))
        wstream = root_ctx.enter_context(tc.tile_pool(name="wstream", bufs=2))
        tp2_psum = root_ctx.enter_context(tc.tile_pool(name="tp2", bufs=2,
                                                       space="PSUM"))
        o3_psum = root_ctx.enter_context(tc.tile_pool(name="o3p", bufs=2,
                                                      space="PSUM"))

        # free-dim biases broadcast across partitions
        bproj_sb = mlp.tile([P, C], BF16)
        nc.gpsimd.dma_start(out=bproj_sb[:], in_=bproj[None, :].to_broadcast((P, C)))
        bmlp_sb = mlp.tile([P, C], BF16)
        nc.gpsimd.dma_start(out=bmlp_sb[:], in_=bmlp[None, :].to_broadcast((P, C)))
        h2T = mlp.tile([P, 8, MLP_TOK], BF16)
        aT = mlp.tile([P, FF // P, MLP_TOK], BF16)
        x1_t = {}
        for half in (1, 0):
            for j in (2 * half, 2 * half + 1):
                rs_sb = mlp.tile([P, C], BF16, name=f"rs_sb{j}", tag="rs_sb")
                nc.sync.dma_start(out=rs_sb[:], in_=rs_out[j * P:(j + 1) * P, :])
                xr = mlp.tile([P, C], F32, name=f"xr{j}", tag="xr")
                nc.sync.dma_start(out=xr[:], in_=x_res[j * P:(j + 1) * P, :])
                x1 = mlp.tile([P, C], F32, name=f"x1_{j}")
                nc.vector.tensor_tensor(out=x1[:], in0=xr[:], in1=rs_sb[:],
                                        op=ALU.add)
                nc.vector.tensor_tensor(out=x1[:], in0=x1[:], in1=bproj_sb[:],
                                        op=ALU.add)
                x1_t[j] = x1
                h2 = mlp.tile([P, C], BF16, name=f"h2_{j}", tag="h2")
                layernorm_tile(x1[:], h2[:], f"ln2_{j}")
                for fg in range(2):
                    tp = tp2_psum.tile([P, 4, P], BF16, name="h2tp", tag="h2tp")
                    for k in range(4):
                        ft = fg * 4 + k
                        nc.tensor.transpose(tp[:, k, :], h2[:, ft * P:(ft + 1) * P],
                                            ident[:])
                    nc.vector.tensor_copy(
                        h2T[:, fg * 4:(fg + 1) * 4, j * P:(j + 1) * P], tp[:])
            # FC + gelu on this 256-token half
            for mt in range(FF // P):
                wsb = wfc_sb1 if mt < 16 else wfc_sb2
                mt0 = mt if mt < 16 else mt - 16
                ps = o3_psum.tile([P, 256], F32, name="fc_ps", tag="o3_ps")
                for kt in range(8):
                    nc.tensor.matmul(ps[:], wsb[:, kt, mt0 * P:(mt0 + 1) * P],
                                     h2T[:, kt, half * 256:(half + 1) * 256],
                                     start=(kt == 0), stop=(kt == 7))
                nc.scalar.activation(aT[:, mt, half * 256:(half + 1) * 256], ps[:],
                                     AF.Gelu, bias=bfc_sb[:, mt:mt + 1])

        # MLP proj + residual accumulated in place into x1
        for ns in range(4):
            wm_q = wstream.tile([P, FF // P, 256], BF16, name="wm_q", tag="wm_q")
            for kt in range(FF // P):
                nc.sync.dma_start(out=wm_q[:, kt, :],
                                  in_=wmlp[kt * P:(kt + 1) * P,
                                           ns * 256:(ns + 1) * 256])
            for j in range(4):
                ps = o3_psum.tile([P, 256], F32, name="o3_ps", tag="o3_ps")
                for kt in range(FF // P):
                    nc.tensor.matmul(ps[:], aT[:, kt, j * P:(j + 1) * P],
                                     wm_q[:, kt, :],
                                     start=(kt == 0), stop=(kt == FF // P - 1))
                nc.vector.tensor_tensor(out=x1_t[j][:, ns * 256:(ns + 1) * 256],
                                        in0=x1_t[j][:, ns * 256:(ns + 1) * 256],
                                        in1=ps[:], op=ALU.add)
        for j in range(4):
            nc.vector.tensor_tensor(out=x1_t[j][:], in0=x1_t[j][:],
                                    in1=bmlp_sb[:], op=ALU.add)
            nc.sync.dma_start(out=out[j * P:(j + 1) * P, :], in_=x1_t[j][:])

    nc.compile()
    return nc


def _get_nc():
    global _CACHED_NC
    if _CACHED_NC is None:
        _CACHED_NC = _build()
    return _CACHED_NC


def _softplus(x):
    return np.log1p(np.exp(-np.abs(x))) + np.maximum(x, 0.0)


def _bf16(x):
    return np.ascontiguousarray(x.astype(ml_dtypes.bfloat16))


def kernel(x, ln1_w, ln1_b, w_attn, b_attn, w_attn_proj, b_attn_proj,
           threshold, leak, steepness, ln2_w, ln2_b,
           w_fc, b_fc, w_mlp_proj, b_mlp_proj):
    x = np.asarray(x, np.float32)
    f32 = lambda a: np.asarray(a, np.float32)
    ln1_w, ln1_b, w_attn, b_attn = map(f32, (ln1_w, ln1_b, w_attn, b_attn))
    w_attn_proj, b_attn_proj = f32(w_attn_proj), f32(b_attn_proj)
    threshold, leak, steepness = map(f32, (threshold, leak, steepness))
    ln2_w, ln2_b, w_fc, b_fc = map(f32, (ln2_w, ln2_b, w_fc, b_fc))
    w_mlp_proj, b_mlp_proj = f32(w_mlp_proj), f32(b_mlp_proj)

    # fold LN affine into the following matmuls (exact in fp32 algebra)
    wa = w_attn * ln1_w[:, None]
    ba = b_attn + ln1_b @ w_attn
    # fold 1/sqrt(D) into the q columns
    wa = wa.copy()
    wa[:, :C] *= 1.0 / np.sqrt(D)
    ba = ba.copy()
    ba[:C] *= 1.0 / np.sqrt(D)
    wf = w_fc * ln2_w[:, None]
    bf = b_fc + ln2_b @ w_fc

    # per-head LIF constants
    st = _softplus(steepness)
    lk = 1.0 / (1.0 + np.exp(-leak))
    th = np.abs(threshold) * 0.1

    wf_b = _bf16(wf)
    wm_b = _bf16(w_mlp_proj)

    in_maps = []
    for c in range(N_CORES):
        b = c // GROUP
        r = c % GROUP
        h0 = r * HL * D  # first local head feature col
        cols = (list(range(h0, h0 + LC))
                + list(range(C + h0, C + h0 + LC))
                + list(range(2 * C + h0, 2 * C + h0 + LC)))
        wqkv_local = _bf16(wa[:, cols])
        bqkv_local = np.ascontiguousarray(ba[cols], dtype=np.float32)
        wproj_local = _bf16(w_attn_proj[h0:h0 + LC, :])
        hsl = slice(r * HL, (r + 1) * HL)
        lif_local = np.stack([
            st[hsl] / 2.0,
            -(st[hsl] * th[hsl]) / 2.0,
            0.5 * (1.0 - lk[hsl]),
            0.5 * (1.0 + lk[hsl]),
        ]).astype(np.float32)
        x_b_core = _bf16(x[b])
        # MLP-phase tokens: RS chunk qs gives rank r rows qs*512+r*128..+128
        x_res_core = np.ascontiguousarray(np.concatenate(
            [x[b][qs * 512 + r * P: qs * 512 + (r + 1) * P] for qs in range(4)]))
        in_maps.append({
            "x_b": x_b_core,
            "x_res": x_res_core,
            "wqkv": wqkv_local,
            "bqkv": bqkv_local,
            "wproj": wproj_local,
            "bproj": b_attn_proj,
            "wfc": wf_b,
            "bfc": bf.astype(np.float32),
            "wmlp": wm_b,
            "bmlp": b_mlp_proj,
            "lif": lif_local,
        })

    global _last_in_maps
    _last_in_maps = in_maps
    nc = _get_nc()
    res = run_bass_kernel_spmd(nc, in_maps, list(range(N_CORES)))

    out = np.empty((B, T, C), np.float32)
    for c in range(N_CORES):
        b = c // GROUP
        r = c % GROUP
        for qs in range(4):
            out[b, qs * 512 + r * P: qs * 512 + (r + 1) * P, :] = \
                res.results[c]["out"][qs * P:(qs + 1) * P]
    return out
